# revision 6
# baseline (speedup 1.0000x reference)
"""CrossPlaneMixer Trainium2 kernel.

Problem: three 5D "plane" tensors (B=2, C=64) at mixed resolutions:
  dh: [2,64,64,64,16]  (full D,H; small W)
  dw: [2,64,64,16,64]  (full D,W; small H)
  hw: [2,64,16,64,64]  (full H,W; small D)
Each plane is mean-reduced along its small axis, the resulting summaries are
adaptively pooled + broadcast into the other planes' grids, concatenated on
the channel axis (3C=192), and mixed by a 1x1x1 conv (w: [64,192], bias [64]).

Distribution: 8 NeuronCores. dh/dw are sharded over D (8 slices of 8),
hw over H.  SBUF partition dim = (b=2 x c=64) = 128 everywhere.

The conv is computed as 3 accumulating PSUM matmuls per tile:
  term1: the plane itself              (rhs = streamed input tile)
  term2/term3: pooled summaries        (rhs = small SBUF-resident tables,
                                        broadcast via stride-0 AP dims)
Weights are block-diagonal [128,128] (the same 64x64 block for both batch
halves) so a single full-array K=128 matmul covers both batches.  Matmul
dtype is float32r (1 cycle/row on PE at N=512 vs 4 for plain fp32).
PSUM->SBUF eviction + per-channel bias runs on ScalarE (activation Identity).

The small pooled summaries (<0.3% of reference FLOPs) are precomputed on the
host and passed per-core, so cores need no cross-core communication.
"""

import os
from contextlib import ExitStack

import numpy as np

import concourse.bass as bass
import concourse.tile as tile
from concourse import bacc, mybir
import concourse.bass_utils as bass_utils

B, C, FULL, SMALL = 2, 64, 64, 16
NCORES = 8
DSH = FULL // NCORES  # 8: per-core slice of D (dh,dw) or H (hw)
P = B * C  # 128 partitions = (b, c)

F32 = mybir.dt.float32
F32R = mybir.dt.float32r

# module-level cache: (nc, ) built+compiled once per process
_BUILT = None
LAST_RESULTS = None  # BassKernelResults of the most recent run (for test.py)


def _build():
    nc = bacc.Bacc(
        "TRN2",
        target_bir_lowering=False,
        debug=False,
        enable_asserts=False,
        num_devices=NCORES,
    )

    # ---- per-core DRAM I/O ----
    x_dh = nc.dram_tensor("x_dh", [B, C, DSH, FULL, SMALL], F32R, kind="ExternalInput")
    x_dw = nc.dram_tensor("x_dw", [B, C, DSH, SMALL, FULL], F32R, kind="ExternalInput")
    x_hw = nc.dram_tensor("x_hw", [B, C, SMALL, DSH, FULL], F32R, kind="ExternalInput")
    # block-diagonal transposed weights: [9, 128, 128] (3 planes x 3 source blocks)
    wts = nc.dram_tensor("wts", [9, P, P], F32R, kind="ExternalInput")
    bia = nc.dram_tensor("bia", [3, P], F32, kind="ExternalInput")
    # pooled summary tables, already laid out [(b c), ...]
    g1 = nc.dram_tensor("g1", [P, FULL, SMALL], F32R, kind="ExternalInput")  # poolW(sum_hw)[h, w16]
    g2 = nc.dram_tensor("g2", [P, SMALL, FULL], F32R, kind="ExternalInput")  # poolH(sum_hw)[h16, w]
    g3 = nc.dram_tensor("g3", [P, SMALL, DSH], F32R, kind="ExternalInput")   # poolD(sum_dh)[d16, h_loc]
    g4 = nc.dram_tensor("g4", [P, SMALL, FULL], F32R, kind="ExternalInput")  # poolD(sum_dw)[d16, w]
    l1 = nc.dram_tensor("l1", [P, DSH, SMALL], F32R, kind="ExternalInput")   # poolW(sum_dw)[d_loc, w16]
    l2 = nc.dram_tensor("l2", [P, DSH, SMALL], F32R, kind="ExternalInput")   # poolH(sum_dh)[d_loc, h16]

    y_dh = nc.dram_tensor("y_dh", [B, C, DSH, FULL, SMALL], F32, kind="ExternalOutput")
    y_dw = nc.dram_tensor("y_dw", [B, C, DSH, SMALL, FULL], F32, kind="ExternalOutput")
    y_hw = nc.dram_tensor("y_hw", [B, C, SMALL, DSH, FULL], F32, kind="ExternalOutput")

    with tile.TileContext(nc) as tc, ExitStack() as ctx:
        cpool = ctx.enter_context(tc.tile_pool(name="const", bufs=1))
        inp = ctx.enter_context(tc.tile_pool(name="inp", bufs=4))
        psp = ctx.enter_context(tc.tile_pool(name="psum", bufs=2, space="PSUM"))
        outp = ctx.enter_context(tc.tile_pool(name="outp", bufs=3))

        # ---- constants into SBUF ----
        wsb = cpool.tile([P, 9, P], F32R)
        nc.sync.dma_start(wsb[:], wts.ap().rearrange("s p m -> p s m"))
        bsb = cpool.tile([P, 3], F32)
        nc.sync.dma_start(bsb[:], bia.ap().rearrange("s p -> p s"))
        g1sb = cpool.tile([P, FULL, SMALL], F32R)
        nc.sync.dma_start(g1sb[:], g1.ap())
        g2sb = cpool.tile([P, SMALL, FULL], F32R)
        nc.sync.dma_start(g2sb[:], g2.ap())
        g3sb = cpool.tile([P, SMALL, DSH], F32R)
        nc.sync.dma_start(g3sb[:], g3.ap())
        g4sb = cpool.tile([P, SMALL, FULL], F32R)
        nc.sync.dma_start(g4sb[:], g4.ap())
        l1sb = cpool.tile([P, DSH, SMALL], F32R)
        nc.sync.dma_start(l1sb[:], l1.ap())
        l2sb = cpool.tile([P, DSH, SMALL], F32R)
        nc.sync.dma_start(l2sb[:], l2.ap())

        def w_of(plane, s):
            return wsb[:, 3 * plane + s, :]

        NCH = 512  # free elems per matmul (= one PSUM fp32 bank)

        # ---------- plane 0: dh ----------
        # in tile per d-pair: [128, 2, 1024] ((h w) contiguous); 4 tiles
        xv = x_dh.ap().rearrange("b c d h w -> (b c) d (h w)")
        yv = y_dh.ap().rearrange("b c d h w -> (b c) (d h w)")
        for dp in range(DSH // 2):
            tin = inp.tile([P, 2, FULL * SMALL], F32R, tag="in_dh")
            nc.sync.dma_start(tin[:], xv[:, 2 * dp : 2 * dp + 2, :])
            ps = psp.tile([P, 2048], F32)
            for s in range(3):
                lhsT = w_of(0, s)
                for j in range(2):  # d within pair
                    dl = 2 * dp + j
                    for n in range(2):  # 512-chunk: h in [32n, 32n+32)
                        if s == 0:
                            rhs = tin[:, j, NCH * n : NCH * (n + 1)]
                        elif s == 1:
                            # l1[dl, w16] broadcast over h (32 rows of chunk)
                            rhs = l1sb[:, 2 * dp + j : 2 * dp + j + 1, :].broadcast_to(
                                [P, 32, SMALL]
                            )
                        else:
                            rhs = g1sb[:, 32 * n : 32 * (n + 1), :]
                        nc.tensor.matmul(
                            ps[:, (j * 2 + n) * NCH : (j * 2 + n + 1) * NCH],
                            lhsT,
                            rhs,
                            start=(s == 0),
                            stop=(s == 2),
                        )
            osb = outp.tile([P, 2048], F32, tag="out_dh")
            nc.scalar.activation(
                osb[:], ps[:], mybir.ActivationFunctionType.Identity,
                bias=bsb[:, 0:1],
            )
            nc.sync.dma_start(yv[:, dp * 2048 : (dp + 1) * 2048], osb[:])

        # ---------- plane 1: dw ----------
        xv = x_dw.ap().rearrange("b c d h w -> (b c) d (h w)")
        yv = y_dw.ap().rearrange("b c d h w -> (b c) (d h w)")
        for dp in range(DSH // 2):
            tin = inp.tile([P, 2, SMALL * FULL], F32R, tag="in_dw")
            nc.sync.dma_start(tin[:], xv[:, 2 * dp : 2 * dp + 2, :])
            ps = psp.tile([P, 2048], F32)
            for s in range(3):
                lhsT = w_of(1, s)
                for j in range(2):
                    dl = 2 * dp + j
                    for n in range(2):  # chunk: h16 in [8n, 8n+8)
                        if s == 0:
                            rhs = tin[:, j, NCH * n : NCH * (n + 1)]
                        elif s == 1:
                            # l2[dl, h16] chunk, broadcast over w (inner 64)
                            rhs = (
                                l2sb[:, dl, 8 * n : 8 * (n + 1)]
                                .unsqueeze(2)
                                .broadcast_to([P, 8, FULL])
                            )
                        else:
                            rhs = g2sb[:, 8 * n : 8 * (n + 1), :]
                        nc.tensor.matmul(
                            ps[:, (j * 2 + n) * NCH : (j * 2 + n + 1) * NCH],
                            lhsT,
                            rhs,
                            start=(s == 0),
                            stop=(s == 2),
                        )
            osb = outp.tile([P, 2048], F32, tag="out_dw")
            nc.scalar.activation(
                osb[:], ps[:], mybir.ActivationFunctionType.Identity,
                bias=bsb[:, 1:2],
            )
            nc.sync.dma_start(yv[:, dp * 2048 : (dp + 1) * 2048], osb[:])

        # ---------- plane 2: hw ----------
        # in tile per h-pair: [128, 16, 2, 64] (d strided, (h w) 512B lines)
        xv = x_hw.ap().rearrange("b c d h w -> (b c) d h w")
        yv = y_hw.ap().rearrange("b c d h w -> (b c) d h w")
        for hp in range(DSH // 2):
            tin = inp.tile([P, SMALL, 2, FULL], F32R, tag="in_hw")
            nc.sync.dma_start(tin[:], xv[:, :, 2 * hp : 2 * hp + 2, :])
            ps = psp.tile([P, 2048], F32)
            for s in range(3):
                lhsT = w_of(2, s)
                for n in range(4):  # chunk: d in [4n, 4n+4)
                    if s == 0:
                        rhs = tin[:, 4 * n : 4 * (n + 1), :, :]
                    elif s == 1:
                        # g3[d16, h_loc] chunk, broadcast over w
                        rhs = (
                            g3sb[:, 4 * n : 4 * (n + 1), 2 * hp : 2 * hp + 2]
                            .unsqueeze(3)
                            .broadcast_to([P, 4, 2, FULL])
                        )
                    else:
                        # g4[d16, w] chunk, broadcast over h (middle dim)
                        rhs = (
                            g4sb[:, 4 * n : 4 * (n + 1), :]
                            .unsqueeze(2)
                            .broadcast_to([P, 4, 2, FULL])
                        )
                    nc.tensor.matmul(
                        ps[:, n * NCH : (n + 1) * NCH],
                        lhsT,
                        rhs,
                        start=(s == 0),
                        stop=(s == 2),
                    )
            osb = outp.tile([P, SMALL, 2, FULL], F32, tag="out_hw")
            nc.scalar.activation(
                osb[:].rearrange("p a b c -> p (a b c)"),
                ps[:],
                mybir.ActivationFunctionType.Identity,
                bias=bsb[:, 2:3],
            )
            nc.sync.dma_start(yv[:, :, 2 * hp : 2 * hp + 2, :], osb[:])

    nc.compile()
    return nc


def _round_f32r(x):
    """Round fp32 to the fp32r format (11-bit mantissa, low 12 bits zero).

    Matches walrus' fp32_to_fp32r: round-to-nearest-even at bit 12. The BIR
    verifier requires fp32r matmul operands to be pre-rounded; doing it host
    side lets the DMA be a pure byte copy.
    """
    u = np.ascontiguousarray(x, np.float32).view(np.uint32)
    r = (u + np.uint32(0x7FF) + ((u >> np.uint32(12)) & np.uint32(1))) & np.uint32(
        0xFFFFF000
    )
    return r.view(np.float32)


def _pool4(x, axis):
    # exact adaptive mean-pool by 4 along `axis` (64 -> 16)
    shp = list(x.shape)
    shp[axis] = 16
    shp.insert(axis + 1, 4)
    return x.reshape(shp).mean(axis=axis + 1)


def _prep_inputs(dh, dw, hw, w_dh, b_dh, w_dw, b_dw, w_hw, b_hw):
    f32 = np.float32
    dh, dw, hw = (np.ascontiguousarray(a, f32) for a in (dh, dw, hw))

    sum_dh = dh.mean(axis=4)  # [b,c,d,h]
    sum_dw = dw.mean(axis=3)  # [b,c,d,w]
    sum_hw = hw.mean(axis=2)  # [b,c,h,w]

    # fp32r-round everything the TensorEngine consumes
    dh, dw, hw = _round_f32r(dh), _round_f32r(dw), _round_f32r(hw)

    p_wdw = _pool4(sum_dw, 3)  # [b,c,d,16]   dw_in_dh
    p_hdh = _pool4(sum_dh, 3)  # [b,c,d,16]   dh_in_dw
    p_whw = _pool4(sum_hw, 3)  # [b,c,h,16]   hw_in_dh
    p_hhw = _pool4(sum_hw, 2)  # [b,c,16,w]   hw_in_dw
    p_ddh = _pool4(sum_dh, 2)  # [b,c,16,h]   dh_in_hw
    p_ddw = _pool4(sum_dw, 2)  # [b,c,16,w]   dw_in_hw

    # block-diagonal transposed weights [9,128,128]
    wts = np.zeros((9, P, P), f32)
    for pi, w in enumerate((w_dh, w_dw, w_hw)):
        w = np.asarray(w, f32)
        for s in range(3):
            blk = w[:, 64 * s : 64 * (s + 1)].T  # [c_in, o]
            wts[3 * pi + s, 0:64, 0:64] = blk
            wts[3 * pi + s, 64:128, 64:128] = blk
    bia = np.stack(
        [np.concatenate([np.asarray(b, f32)] * 2) for b in (b_dh, b_dw, b_hw)]
    )  # [3,128]

    wts = _round_f32r(wts)
    p_wdw, p_hdh, p_whw, p_hhw, p_ddh, p_ddw = (
        _round_f32r(a) for a in (p_wdw, p_hdh, p_whw, p_hhw, p_ddh, p_ddw)
    )
    g1_ = np.ascontiguousarray(p_whw.reshape(P, FULL, SMALL))
    g2_ = np.ascontiguousarray(p_hhw.reshape(P, SMALL, FULL))
    g4_ = np.ascontiguousarray(p_ddw.reshape(P, SMALL, FULL))

    in_maps = []
    for k in range(NCORES):
        dsl = slice(DSH * k, DSH * (k + 1))
        in_maps.append(
            {
                "x_dh": np.ascontiguousarray(dh[:, :, dsl]),
                "x_dw": np.ascontiguousarray(dw[:, :, dsl]),
                "x_hw": np.ascontiguousarray(hw[:, :, :, dsl, :]),
                "wts": wts,
                "bia": bia,
                "g1": g1_,
                "g2": g2_,
                "g3": np.ascontiguousarray(p_ddh.reshape(P, SMALL, FULL)[:, :, dsl]),
                "g4": g4_,
                "l1": np.ascontiguousarray(p_wdw.reshape(P, FULL, SMALL)[:, dsl, :]),
                "l2": np.ascontiguousarray(p_hdh.reshape(P, FULL, SMALL)[:, dsl, :]),
            }
        )
    return in_maps


def _run(inputs: dict, trace: bool = False):
    global _BUILT, LAST_RESULTS
    if _BUILT is None:
        _BUILT = _build()
    nc = _BUILT

    in_maps = _prep_inputs(**inputs)
    res = bass_utils.run_bass_kernel_spmd(
        nc, in_maps, core_ids=list(range(NCORES)), trace=trace
    )
    LAST_RESULTS = res

    dh_new = np.empty((B, C, FULL, FULL, SMALL), np.float32)
    dw_new = np.empty((B, C, FULL, SMALL, FULL), np.float32)
    hw_new = np.empty((B, C, SMALL, FULL, FULL), np.float32)
    for k in range(NCORES):
        dsl = slice(DSH * k, DSH * (k + 1))
        dh_new[:, :, dsl] = res.results[k]["y_dh"]
        dw_new[:, :, dsl] = res.results[k]["y_dw"]
        hw_new[:, :, :, dsl, :] = res.results[k]["y_hw"]
    return dh_new, dw_new, hw_new


def kernel(**inputs):
    return _run(inputs, trace=bool(os.environ.get("KERNEL_TRACE")))


# revision 7
# speedup vs baseline: 1.6468x; 1.6468x over previous
"""CrossPlaneMixer Trainium2 kernel.

Problem: three 5D "plane" tensors (B=2, C=64) at mixed resolutions:
  dh: [2,64,64,64,16]  (full D,H; small W)
  dw: [2,64,64,16,64]  (full D,W; small H)
  hw: [2,64,16,64,64]  (full H,W; small D)
Each plane is mean-reduced along its small axis, the summaries are pooled +
broadcast into the other planes' grids, concatenated on channels (3C=192),
and mixed by a 1x1x1 conv (w: [64,192], bias [64]).

Distribution: 8 NeuronCores. dh/dw sharded over D (8 slices of 8), hw over
H.  SBUF partition dim = (b=2 x c=64) = 128 everywhere; a block-diagonal
[128,128] weight (the 64x64 block replicated on both halves) makes a single
full-array K=128 matmul cover both batch items.

Per output tile the conv is 3 accumulating PSUM matmuls:
  term1: the plane itself           (rhs = streamed bf16 input tile)
  term2/term3: pooled summaries     (rhs = small SBUF tables, broadcast via
                                     stride-0 AP dims — no materialization)
Matmul operands are bf16 (fp32 would be 4 cyc/row and double the DMA bytes);
accumulation stays fp32 in PSUM.  Eviction PSUM->SBUF runs on VectorE as
tensor_scalar_add(bias) with bf16 output; outputs are upcast to fp32 on the
host.  Input DMAs issue on the SP HWDGE queue, output DMAs on the ACT HWDGE
queue so a blocked output never stalls the input stream.

The small pooled summaries (<0.3% of reference FLOPs) are precomputed on the
host and passed per-core, so cores need no cross-core communication.
"""

import os
from contextlib import ExitStack

import ml_dtypes
import numpy as np

import concourse.bass as bass
import concourse.tile as tile
from concourse import bacc, mybir
import concourse.bass_utils as bass_utils

B, C, FULL, SMALL = 2, 64, 64, 16
NCORES = 8
DSH = FULL // NCORES  # 8: per-core slice of D (dh,dw) or H (hw)
P = B * C  # 128 partitions = (b, c)

F32 = mybir.dt.float32
BF16 = mybir.dt.bfloat16
NPBF16 = ml_dtypes.bfloat16

NCH = 512  # free elems per matmul = one fp32 PSUM bank

_BUILT = None
LAST_RESULTS = None  # BassKernelResults of the most recent run (for test.py)


def _build():
    nc = bacc.Bacc(
        "TRN2",
        target_bir_lowering=False,
        debug=False,
        enable_asserts=False,
        num_devices=NCORES,
    )

    # ---- per-core DRAM I/O (bf16 on the wire) ----
    x_dh = nc.dram_tensor("x_dh", [B, C, DSH, FULL, SMALL], BF16, kind="ExternalInput")
    x_dw = nc.dram_tensor("x_dw", [B, C, DSH, SMALL, FULL], BF16, kind="ExternalInput")
    x_hw = nc.dram_tensor("x_hw", [B, C, SMALL, DSH, FULL], BF16, kind="ExternalInput")
    # block-diagonal transposed weights: [9, 128, 128] (3 planes x 3 blocks)
    wts = nc.dram_tensor("wts", [9, P, P], BF16, kind="ExternalInput")
    bia = nc.dram_tensor("bia", [3, P], F32, kind="ExternalInput")
    # pooled summary tables, already laid out [(b c), ...]
    g1 = nc.dram_tensor("g1", [P, FULL, SMALL], BF16, kind="ExternalInput")  # poolW(sum_hw)[h,w16]
    g2 = nc.dram_tensor("g2", [P, SMALL, FULL], BF16, kind="ExternalInput")  # poolH(sum_hw)[h16,w]
    g3 = nc.dram_tensor("g3", [P, SMALL, DSH], BF16, kind="ExternalInput")   # poolD(sum_dh)[d16,h_loc]
    g4 = nc.dram_tensor("g4", [P, SMALL, FULL], BF16, kind="ExternalInput")  # poolD(sum_dw)[d16,w]
    l1 = nc.dram_tensor("l1", [P, DSH, SMALL], BF16, kind="ExternalInput")   # poolW(sum_dw)[d_loc,w16]
    l2 = nc.dram_tensor("l2", [P, DSH, SMALL], BF16, kind="ExternalInput")   # poolH(sum_dh)[d_loc,h16]

    y_dh = nc.dram_tensor("y_dh", [B, C, DSH, FULL, SMALL], BF16, kind="ExternalOutput")
    y_dw = nc.dram_tensor("y_dw", [B, C, DSH, SMALL, FULL], BF16, kind="ExternalOutput")
    y_hw = nc.dram_tensor("y_hw", [B, C, SMALL, DSH, FULL], BF16, kind="ExternalOutput")

    with tile.TileContext(nc) as tc, ExitStack() as ctx:
        cpool = ctx.enter_context(tc.tile_pool(name="const", bufs=1))
        inp = ctx.enter_context(tc.tile_pool(name="inp", bufs=4))
        psp = ctx.enter_context(tc.tile_pool(name="psum", bufs=2, space="PSUM"))
        outp = ctx.enter_context(tc.tile_pool(name="outp", bufs=4))

        # ---- constants into SBUF (wts first: the first matmul needs it) ----
        wsb = cpool.tile([P, 9, P], BF16)
        nc.sync.dma_start(wsb[:], wts.ap().rearrange("s p m -> p s m"))
        l1sb = cpool.tile([P, DSH, SMALL], BF16)
        nc.sync.dma_start(l1sb[:], l1.ap())
        g1sb = cpool.tile([P, FULL, SMALL], BF16)
        nc.sync.dma_start(g1sb[:], g1.ap())
        bsb = cpool.tile([P, 3], F32)
        nc.sync.dma_start(bsb[:], bia.ap().rearrange("s p -> p s"))
        l2sb = cpool.tile([P, DSH, SMALL], BF16)
        nc.sync.dma_start(l2sb[:], l2.ap())
        g2sb = cpool.tile([P, SMALL, FULL], BF16)
        nc.sync.dma_start(g2sb[:], g2.ap())
        g3sb = cpool.tile([P, SMALL, DSH], BF16)
        nc.sync.dma_start(g3sb[:], g3.ap())
        g4sb = cpool.tile([P, SMALL, FULL], BF16)
        nc.sync.dma_start(g4sb[:], g4.ap())

        def w_of(plane, s):
            return wsb[:, 3 * plane + s, :]

        def evict(plane, ps, osb_ap):
            # PSUM -> SBUF with per-channel bias, fp32 -> bf16
            nc.vector.tensor_scalar_add(osb_ap, ps[:], bsb[:, plane : plane + 1])

        # ---------- planes 0/1: dh, dw (identical structure) ----------
        # in tile = d-quad [128, 4, 1024] (8KB contiguous lines); 2 per plane
        for plane, x, y in ((0, x_dh, y_dh), (1, x_dw, y_dw)):
            xv = x.ap().rearrange("b c d h w -> (b c) d (h w)")
            yv = y.ap().rearrange("b c d h w -> (b c) (d h w)")
            for q in range(DSH // 4):
                tin = inp.tile([P, 4, 1024], BF16, tag="in")
                nc.sync.dma_start(tin[:], xv[:, 4 * q : 4 * q + 4, :])
                for t in range(2):  # psum tile per d-pair
                    ps = psp.tile([P, 2048], F32)
                    for s in range(3):
                        lhsT = w_of(plane, s)
                        for j in range(2):  # d within pair
                            dl = 4 * q + 2 * t + j
                            for n in range(2):  # 512-chunk
                                if s == 0:
                                    rhs = tin[:, 2 * t + j, NCH * n : NCH * (n + 1)]
                                elif s == 1:
                                    if plane == 0:
                                        # l1[dl, w16] bcast over h (chunk rows)
                                        rhs = l1sb[:, dl : dl + 1, :].broadcast_to(
                                            [P, 32, SMALL]
                                        )
                                    else:
                                        # l2[dl, h16] chunk, bcast over w
                                        rhs = (
                                            l2sb[:, dl, 8 * n : 8 * (n + 1)]
                                            .unsqueeze(2)
                                            .broadcast_to([P, 8, FULL])
                                        )
                                elif plane == 0:
                                    rhs = g1sb[:, 32 * n : 32 * (n + 1), :]
                                else:
                                    rhs = g2sb[:, 8 * n : 8 * (n + 1), :]
                                nc.tensor.matmul(
                                    ps[:, (j * 2 + n) * NCH : (j * 2 + n + 1) * NCH],
                                    lhsT,
                                    rhs,
                                    start=(s == 0),
                                    stop=(s == 2),
                                )
                    osb = outp.tile([P, 2048], BF16, tag="out")
                    evict(plane, ps, osb[:])
                    nc.scalar.dma_start(
                        yv[:, (4 * q + 2 * t) * 1024 : (4 * q + 2 * t + 2) * 1024],
                        osb[:],
                    )

        # ---------- plane 2: hw ----------
        # in tile = h-quad [128, 16, 4, 64] (512B lines); 2 tiles
        xv = x_hw.ap().rearrange("b c d h w -> (b c) d h w")
        yv = y_hw.ap().rearrange("b c d h w -> (b c) d h w")
        for q in range(DSH // 4):
            tin = inp.tile([P, SMALL, 4, FULL], BF16, tag="in")
            nc.sync.dma_start(tin[:], xv[:, :, 4 * q : 4 * q + 4, :])
            for t in range(2):  # psum tile per 8 d's
                ps = psp.tile([P, 2048], F32)
                for s in range(3):
                    lhsT = w_of(2, s)
                    for m in range(4):  # chunk: d in [8t+2m, 8t+2m+2)
                        d0 = 8 * t + 2 * m
                        if s == 0:
                            rhs = tin[:, d0 : d0 + 2, :, :]
                        elif s == 1:
                            # g3[d16, h_loc] chunk, bcast over w
                            rhs = (
                                g3sb[:, d0 : d0 + 2, 4 * q : 4 * q + 4]
                                .unsqueeze(3)
                                .broadcast_to([P, 2, 4, FULL])
                            )
                        else:
                            # g4[d16, w] chunk, bcast over h (middle)
                            rhs = (
                                g4sb[:, d0 : d0 + 2, :]
                                .unsqueeze(2)
                                .broadcast_to([P, 2, 4, FULL])
                            )
                        nc.tensor.matmul(
                            ps[:, m * NCH : (m + 1) * NCH],
                            lhsT,
                            rhs,
                            start=(s == 0),
                            stop=(s == 2),
                        )
                osb = outp.tile([P, 8, 4, FULL], BF16, tag="out")
                evict(2, ps, osb[:].rearrange("p a b c -> p (a b c)"))
                nc.scalar.dma_start(
                    yv[:, 8 * t : 8 * t + 8, 4 * q : 4 * q + 4, :], osb[:]
                )

    nc.compile()
    return nc


def _pool4(x, axis):
    # exact adaptive mean-pool by 4 along `axis` (64 -> 16)
    shp = list(x.shape)
    shp[axis] = 16
    shp.insert(axis + 1, 4)
    return x.reshape(shp).mean(axis=axis + 1)


def _prep_inputs(dh, dw, hw, w_dh, b_dh, w_dw, b_dw, w_hw, b_hw):
    f32 = np.float32
    dh, dw, hw = (np.ascontiguousarray(a, f32) for a in (dh, dw, hw))

    sum_dh = dh.mean(axis=4)  # [b,c,d,h]
    sum_dw = dw.mean(axis=3)  # [b,c,d,w]
    sum_hw = hw.mean(axis=2)  # [b,c,h,w]

    p_wdw = _pool4(sum_dw, 3)  # [b,c,d,16]   dw_in_dh
    p_hdh = _pool4(sum_dh, 3)  # [b,c,d,16]   dh_in_dw
    p_whw = _pool4(sum_hw, 3)  # [b,c,h,16]   hw_in_dh
    p_hhw = _pool4(sum_hw, 2)  # [b,c,16,w]   hw_in_dw
    p_ddh = _pool4(sum_dh, 2)  # [b,c,16,h]   dh_in_hw
    p_ddw = _pool4(sum_dw, 2)  # [b,c,16,w]   dw_in_hw

    bf = lambda a: np.ascontiguousarray(a, NPBF16)
    dh, dw, hw = bf(dh), bf(dw), bf(hw)

    # block-diagonal transposed weights [9,128,128]
    wts = np.zeros((9, P, P), NPBF16)
    for pi, w in enumerate((w_dh, w_dw, w_hw)):
        w = np.asarray(w, f32)
        for s in range(3):
            blk = bf(w[:, 64 * s : 64 * (s + 1)].T)  # [c_in, o]
            wts[3 * pi + s, 0:64, 0:64] = blk
            wts[3 * pi + s, 64:128, 64:128] = blk
    bia = np.stack(
        [np.concatenate([np.asarray(b, f32)] * 2) for b in (b_dh, b_dw, b_hw)]
    )  # [3,128]

    g1_ = bf(p_whw.reshape(P, FULL, SMALL))
    g2_ = bf(p_hhw.reshape(P, SMALL, FULL))
    g4_ = bf(p_ddw.reshape(P, SMALL, FULL))

    in_maps = []
    for k in range(NCORES):
        dsl = slice(DSH * k, DSH * (k + 1))
        in_maps.append(
            {
                "x_dh": np.ascontiguousarray(dh[:, :, dsl]),
                "x_dw": np.ascontiguousarray(dw[:, :, dsl]),
                "x_hw": np.ascontiguousarray(hw[:, :, :, dsl, :]),
                "wts": wts,
                "bia": bia,
                "g1": g1_,
                "g2": g2_,
                "g3": bf(p_ddh.reshape(P, SMALL, FULL)[:, :, dsl]),
                "g4": g4_,
                "l1": bf(p_wdw.reshape(P, FULL, SMALL)[:, dsl, :]),
                "l2": bf(p_hdh.reshape(P, FULL, SMALL)[:, dsl, :]),
            }
        )
    return in_maps


def _run(inputs: dict, trace: bool = False):
    global _BUILT, LAST_RESULTS
    if _BUILT is None:
        _BUILT = _build()
    nc = _BUILT

    in_maps = _prep_inputs(**inputs)
    res = bass_utils.run_bass_kernel_spmd(
        nc, in_maps, core_ids=list(range(NCORES)), trace=trace
    )
    LAST_RESULTS = res

    dh_new = np.empty((B, C, FULL, FULL, SMALL), np.float32)
    dw_new = np.empty((B, C, FULL, SMALL, FULL), np.float32)
    hw_new = np.empty((B, C, SMALL, FULL, FULL), np.float32)
    for k in range(NCORES):
        dsl = slice(DSH * k, DSH * (k + 1))
        dh_new[:, :, dsl] = res.results[k]["y_dh"].astype(np.float32)
        dw_new[:, :, dsl] = res.results[k]["y_dw"].astype(np.float32)
        hw_new[:, :, :, dsl, :] = res.results[k]["y_hw"].astype(np.float32)
    return dh_new, dw_new, hw_new


def kernel(**inputs):
    return _run(inputs, trace=bool(os.environ.get("KERNEL_TRACE")))
